# revision 38
# baseline (speedup 1.0000x reference)
"""Causal self-attention (B=4, T=2048, C=1024, H=16) on 8 trn2 NeuronCores.

Sharding: tensor-parallel over heads x data-parallel over batch.
Core c handles batch b=c//2 and head group g=c%2 (8 heads each).
Each core computes qkv projection for its heads, causal attention, and a
partial output projection; the host sums the two partial yT per batch and
adds the output bias.

Device dataflow is feature-major ("transposed") end to end:
  qkT[f, t]   = Wqk.T @ xT          (f = head-pair-blocked q/k features)
  scoresT[k, q] = kT.T @ qT         per head, k-tile=128 x q-tile=512
  e = exp(scoresT/8), causal-masked via affine_select
  avT[d(+1), q] += [v|1].T @ e      ones-column gives softmax denominator
  aoT = avT[0:64] * (1/avT[64]) broadcast (PE outer-product broadcast)
  yT_partial = Wo.T @ aoT
No transposes are needed anywhere; the host transposes x and y (free).
Heads are packed two per 128-partition block (even head at partitions 0-63,
odd at 64-127) so the K=64 score matmuls of a pair run row-tiled
concurrently in the PE array.
"""

import os
import threading
from contextlib import ExitStack

import ml_dtypes
import numpy as np

import concourse.bass as bass
from concourse import bacc
import concourse.mybir as mybir
import concourse.tile as tile
from concourse.bass_utils import run_bass_kernel_spmd

B, T, C = 4, 2048, 1024
H, D = 16, 64
NCORES = 8
HL = 8                 # heads per core
NPAIR = HL // 2        # head pairs per core
CQK = 2 * HL * D       # 1024 local q+k features
CV = HL * D            # 512 local v features
TQ = 512               # query tile (PSUM bank limit for f32)
NQT = T // TQ          # 4
TK = 128               # key tile (PSUM partition limit)
NKT = T // TK          # 16
KO = C // 128          # 8 contraction tiles over C
F32 = mybir.dt.float32
BF16 = mybir.dt.bfloat16

# float32r: full-precision fp32 data, fast PE streaming mode (1 cycle/row at
# N>=256 vs 4 for plain float32).
MM_DT = {
    "f32r": mybir.dt.float32r,
    "f32": mybir.dt.float32,
}[os.environ.get("ATTN_MM_DT", "f32r")]


def r(ap):
    """View an fp32 AP as the matmul input dtype (float32r needs producers to
    write through an fp32r-typed AP so the BIR verifier sees rounded data)."""
    if MM_DT == F32 or ap.dtype != F32:
        return ap
    return ap.bitcast(MM_DT)


def _mm(nc, out, lhsT, rhs, start=True, stop=True):
    nc.tensor.matmul(out, r(lhsT), r(rhs), start=start, stop=stop)


def build_program():
    nc = bacc.Bacc(None)
    xT = nc.declare_dram_parameter("xT", [C, T], BF16, isOutput=False)
    wqk = nc.declare_dram_parameter("wqk", [C, CQK], BF16, isOutput=False)
    bqk = nc.declare_dram_parameter("bqk", [CQK], F32, isOutput=False)
    wv = nc.declare_dram_parameter("wv", [C, CV], BF16, isOutput=False)
    bv = nc.declare_dram_parameter("bv", [CV], F32, isOutput=False)
    wo = nc.declare_dram_parameter("wo", [CV, C], BF16, isOutput=False)
    yT = nc.declare_dram_parameter("yT", [C, T], BF16, isOutput=True)

    with ExitStack() as ctx:
        ctx.enter_context(nc.allow_low_precision(reason="fp32r matmul inputs"))
        tc = ctx.enter_context(tile.TileContext(nc))
        persist = ctx.enter_context(tc.tile_pool(name="persist", bufs=1))
        p2 = ctx.enter_context(tc.tile_pool(name="p2", bufs=3))
        pw = ctx.enter_context(tc.tile_pool(name="pw", bufs=1))
        px = ctx.enter_context(tc.tile_pool(name="px", bufs=2))
        ps = ctx.enter_context(tc.tile_pool(name="ps", bufs=2, space="PSUM"))
        ps_av = ctx.enter_context(tc.tile_pool(name="ps_av", bufs=2, space="PSUM"))
        dram = ctx.enter_context(tc.tile_pool(name="dram", bufs=2, space="DRAM"))

        # q/k features, head-pair blocked: block m<4 = q of pair m
        # (even head partitions 0-63, odd 64-127), block 4+m = k of pair m.
        # One tile per 512-token chunk so chunk writes and attention reads
        # of different chunks never false-serialize (deps are per-tile).
        qkTs = [persist.tile([128, 8, TQ], BF16, name=f"qkT{c}")
                for c in range(NQT)]
        # v with ones column for the softmax denominator: [tok, kt, head, d+1]
        v_augs = [persist.tile([128, TQ // TK, HL, D + 1], BF16,
                               name=f"vaug{c}") for c in range(NQT)]
        bqk_sb = persist.tile([128, 8], F32)
        bv_row = persist.tile([1, CV], F32)
        bvb_sb = persist.tile([128, CV], F32)    # v bias broadcast over tokens
        ones_sb = persist.tile([128, 128], F32)
        wo_sb = persist.tile([128, 4, C], BF16)
        # normalized attention output, one tile per head pair (per-tile deps:
        # the projection's per-ko reads then only wait on that pair's norm)
        aoTs = [persist.tile([128, T], BF16, name=f"aoT{p}")
                for p in range(NPAIR)]

        ones_f32 = persist.tile([128, 128], F32)
        nc.vector.memset(ones_f32, 1.0)
        nc.vector.tensor_copy(out=r(ones_sb[:]), in_=ones_f32)
        for c in range(NQT):
            nc.vector.tensor_copy(
                out=v_augs[c][:, :, :, D : D + 1],
                in_=ones_f32[:, 0 : (TQ // TK) * HL].rearrange(
                    "p (a b c) -> p a b c", a=TQ // TK, b=HL))
        nc.sync.dma_start(out=bqk_sb, in_=bqk[:].rearrange("(m p) -> p m", p=128))
        nc.sync.dma_start(out=r(bv_row[:]), in_=r(bv[:].unsqueeze(0)))

        xT_r = xT[:].rearrange("(ko p) t -> p ko t", p=128)
        wv_r = wv[:].rearrange("(ko p) f -> p ko f", p=128)
        wqk_r = wqk[:].rearrange("(ko p) f -> p ko f", p=128)
        # chunk-0 x and the v weights load first (ko-halves for finer deps)
        # so the first v matmuls start as early as possible.
        KH = KO // 2
        xt0 = [px.tile([128, KH, TQ], BF16, name=f"xt0_{h}", tag=f"xt{h}")
               for h in range(2)]
        wv_sb = [pw.tile([128, KH, CV], BF16, name=f"wv_{h}", tag=f"wv{h}")
                 for h in range(2)]
        # two parallel DMA queues for the startup loads. The attention
        # critical chain needs the first pair's q/k blocks = xt (sync queue)
        # + wqk (gpsimd queue), both in ko-halves, so the first q/k matmuls
        # start at ~1MB queue depth and the full chain (qk -> bias ->
        # scores -> exp -> av) is rolling by ~10us. wv lands last on sync;
        # the v matmuls fill the PE behind the early attention steps.
        wqk_sb = [pw.tile([128, KH, CQK], BF16, name=f"wqk_{h}", tag=f"wqk{h}")
                  for h in range(2)]
        for h in range(2):
            nc.sync.dma_start(out=r(xt0[h][:]),
                              in_=r(xT_r[:, h * KH : (h + 1) * KH, 0:TQ]))
            nc.gpsimd.dma_start(out=r(wqk_sb[h][:]),
                                in_=r(wqk_r[:, h * KH : (h + 1) * KH, :]))
        for h in range(2):
            nc.sync.dma_start(out=r(wv_sb[h][:]),
                              in_=r(wv_r[:, h * KH : (h + 1) * KH, :]))

        # v-bias broadcast over the 128 token partitions via K=1 outer product
        bvb_ps = ps.tile([128, CV], F32, tag="s")
        _mm(nc, bvb_ps, ones_sb[0:1, :], bv_row)
        nc.vector.tensor_copy(out=bvb_sb, in_=bvb_ps)

        def qkv_chunk_items(ch, xt, split_v=False):
            """Per-chunk QKV work, as one closure per matmul group."""
            t0 = ch * TQ

            def v_mt(mt, kos=range(KO), acc_in=None):
                def f():
                    acc = acc_in or ps.tile([128, CV], F32, tag="s")
                    for ko in kos:
                        _mm(nc, acc,
                            xt[ko // KH][:, ko % KH, mt * TK : (mt + 1) * TK],
                            wv_sb[ko // KH][:, ko % KH, :],
                            start=ko == 0, stop=ko == KO - 1)
                    if kos[-1] == KO - 1:
                        nc.vector.tensor_add(
                            out=v_augs[ch][:, mt, :, 0:D],
                            in0=acc.rearrange("p (h d) -> p h d", d=D),
                            in1=bvb_sb.rearrange("p (h d) -> p h d", d=D))
                    return acc
                return f

            def qk_m(m, kos=range(KO), acc_in=None):
                def f():
                    acc = acc_in or ps.tile([128, TQ], F32, tag="s")
                    for ko in kos:
                        _mm(nc, acc,
                            wqk_sb[ko // KH][:, ko % KH, m * 128 : (m + 1) * 128],
                            xt[ko // KH][:, ko % KH, :],
                            start=ko == 0, stop=ko == KO - 1)
                    if kos[-1] == KO - 1:
                        nc.vector.tensor_scalar_add(
                            out=qkTs[ch][:, m, :], in0=acc,
                            scalar1=bqk_sb[:, m : m + 1])
                    return acc
                return f

            if split_v:
                # chunk-0 prologue: half-contraction interleave (A = ko
                # h0 halves, which arrive first). The first pair's q/k
                # blocks (0 and 4) go first: they start the attention
                # critical chain. v fills the PE behind them. At most two
                # accumulation groups are open at any point (the PSUM "s"
                # pool has two slots).
                accs = {}
                A, Bk = list(range(KH)), list(range(KH, KO))
                def gA(mk, i):
                    return lambda: accs.__setitem__((mk, i), (qk_m if mk == "q"
                        else v_mt)(i, kos=A)())
                def gB(mk, i):
                    return lambda: (qk_m if mk == "q" else v_mt)(
                        i, kos=Bk, acc_in=accs[(mk, i)])()
                seq = [("q", 0), ("q", 4), ("q", 0), ("q", 4),
                       ("v", 0), ("v", 0), ("v", 1), ("v", 1),
                       ("v", 2), ("v", 2), ("v", 3), ("v", 3)]
                seen = set()
                items = []
                for mk, i in seq:
                    if (mk, i) in seen:
                        items.append(gB(mk, i))
                    else:
                        seen.add((mk, i))
                        items.append(gA(mk, i))
                return items, [qk_m(m) for m in (1, 5, 2, 6, 3, 7)]
            return [v_mt(mt) for mt in range(TQ // TK)] + \
                   [qk_m(m) for m in range(8)]

        def load_chunk(ch):
            xt = [px.tile([128, KH, TQ], BF16, name=f"xt_{ch}_{h}", tag=f"xt{h}")
                  for h in range(2)]
            t0 = ch * TQ
            for h in range(2):
                nc.sync.dma_start(
                    out=r(xt[h][:]),
                    in_=r(xT_r[:, h * KH : (h + 1) * KH, t0 : t0 + TQ]))
            return xt

        # chunk 0 prologue: the first pair's q/k blocks and v run dense;
        # the other six q/k blocks spread into the attention stream (pair p
        # of qt0 only needs blocks p and 4+p, which land pairs ahead).
        pre0, rest0 = qkv_chunk_items(0, xt0, split_v=True)
        for f in pre0:
            f()
        # out-proj weights are not needed until much later; load them now so
        # the DMA does not compete with the startup x/wv/wqk loads.
        nc.sync.dma_start(out=wo_sb, in_=wo[:].rearrange("(ko p) f -> p ko f", p=128))

        def make_norm(pair, q0, av_E, av_O, pe_bcast=False):
            def norm():
                if pe_bcast:
                    # tail variant: broadcast denominators with a K=1 PE outer
                    # product (PE is idle here) instead of the DRAM bounce,
                    # skipping two DMA-completion latencies.
                    stage = p2.tile([128, 2 * TQ], F32, tag="rec", bufs=2)
                    nc.vector.tensor_copy(out=r(stage[64:65, 0:TQ]),
                                          in_=av_E[D : D + 1, :])
                    nc.vector.tensor_copy(out=r(stage[64:65, TQ : 2 * TQ]),
                                          in_=av_O[D : D + 1, :])
                    bc_ps = ps.tile([64, 2 * TQ], F32, tag="s")
                    _mm(nc, bc_ps[:, 0:TQ], ones_sb[64:65, 0:64],
                        stage[64:65, 0:TQ])
                    _mm(nc, bc_ps[:, TQ : 2 * TQ], ones_sb[64:65, 0:64],
                        stage[64:65, TQ : 2 * TQ])
                    bc_sb = p2.tile([64, 2 * TQ], F32, tag="recbc", bufs=2)
                    nc.vector.reciprocal_approx_fast(out=bc_sb, in_=bc_ps)
                    nc.vector.tensor_mul(
                        out=aoTs[pair][0:64, q0 : q0 + TQ],
                        in0=av_E[0:D, :], in1=bc_sb[:, 0:TQ])
                    ao_tmp = p2.tile([64, TQ], BF16, tag="aotmp")
                    nc.vector.tensor_mul(out=ao_tmp, in0=av_O[0:D, :],
                                         in1=bc_sb[:, TQ : 2 * TQ])
                    nc.sync.dma_start(out=aoTs[pair][64:128, q0 : q0 + TQ],
                                      in_=ao_tmp)
                    return
                # denominators (av row D) -> SBUF -> DRAM -> 0-step-partition
                # DMA fans them over the 64 d-partitions; the reciprocal then
                # runs 64-lane-parallel at partition 0 (reciprocal_approx_fast
                # misbehaves at base partition 64).
                stage = p2.tile([128, 2 * TQ], F32, tag="rec", bufs=2)
                nc.vector.tensor_copy(out=stage[64:65, 0:TQ],
                                      in_=av_E[D : D + 1, :])
                nc.vector.tensor_copy(out=stage[64:65, TQ : 2 * TQ],
                                      in_=av_O[D : D + 1, :])
                dr = dram.tile([1, 2 * TQ], F32, tag="drrec")
                # the gpsimd DMA queue is idle after startup; the sync queue
                # carries x-chunk loads and y writes which would delay the
                # norm chain at q-tile boundaries
                nc.gpsimd.dma_start(out=dr, in_=stage[64:65, :])
                den_bc = p2.tile([64, 2 * TQ], F32, tag="bc", bufs=2)
                nc.gpsimd.dma_start(out=den_bc,
                                    in_=dr[:].to_broadcast([64, 2 * TQ]))
                bc_sb = p2.tile([64, 2 * TQ], F32, tag="recbc", bufs=2)
                nc.vector.reciprocal_approx_fast(out=bc_sb, in_=den_bc)
                nc.vector.tensor_mul(
                    out=aoTs[pair][0:64, q0 : q0 + TQ],
                    in0=av_E[0:D, :], in1=bc_sb[:, 0:TQ])
                ao_tmp = p2.tile([64, TQ], BF16, tag="aotmp")
                nc.vector.tensor_mul(out=ao_tmp, in0=av_O[0:D, :],
                                     in1=bc_sb[:, TQ : 2 * TQ])
                # odd head lives at partitions 64-127: DMA does the hop
                nc.gpsimd.dma_start(out=aoTs[pair][64:128, q0 : q0 + TQ],
                                    in_=ao_tmp)
            return norm

        def make_proj(q0, ko_order=(0, 1, 2, 3)):
            def proj_m(m):
                def f():
                    acc = ps.tile([128, TQ], F32, tag="s")
                    for i, ko in enumerate(ko_order):
                        _mm(nc, acc, wo_sb[:, ko, m * 128 : (m + 1) * 128],
                            aoTs[ko][:, q0 : q0 + TQ], start=i == 0, stop=i == 3)
                    y_sb = p2.tile([128, TQ], BF16, tag="ysb", bufs=2)
                    nc.vector.tensor_copy(out=y_sb, in_=acc)
                    nc.sync.dma_start(
                        out=yT[m * 128 : (m + 1) * 128, q0 : q0 + TQ], in_=y_sb)
                return f
            return [proj_m(m) for m in range(8)]

        # Pending PE work spread one item per kt into the ACT-paced attention
        # stream: next chunk's QKV groups (deadline: before the next q-tile)
        # and the previous q-tile's projection (needs this qt's norms done).
        q_chunk = list(rest0)
        q_proj = []

        for qt in range(NQT):
            q0 = qt * TQ
            nkt = (q0 + TQ) // TK  # causal: only k-tiles with k0 <= q0+TQ-1
            if qt + 1 < NQT:
                q_chunk.extend(qkv_chunk_items(qt + 1, load_chunk(qt + 1)))
            ktg = 0
            pair_order = (1, 2, 3, 0) if qt == NQT - 1 else range(NPAIR)
            for pair in pair_order:
                qE = qkTs[qt][0:64, pair, :]
                qO = qkTs[qt][64:128, pair, :]
                av_E = ps_av.tile([D + 1, TQ], F32, tag="avE")
                av_O = ps_av.tile([D + 1, TQ], F32, tag="avO")

                def av_mms(e_sb, kt, c0):
                    vc, vk = kt // (TQ // TK), kt % (TQ // TK)
                    _mm(nc, av_E[:, c0:TQ],
                        v_augs[vc][:, vk, 2 * pair, :], e_sb[:, c0:TQ],
                        start=kt == 0, stop=kt == nkt - 1)
                    _mm(nc, av_O[:, c0:TQ],
                        v_augs[vc][:, vk, 2 * pair + 1, :],
                        e_sb[:, TQ + c0 : 2 * TQ],
                        start=kt == 0, stop=kt == nkt - 1)

                # av matmuls deferred so the next scores sit ahead of
                # av(kt) in the PE queue: the PE computes scores while ACT
                # exps the previous block. Two kts of slack in the late
                # q-tiles, where there is little other PE work per kt and
                # one kt of slack is shorter than the exp latency.
                av_depth = 2 if qt >= 2 else 1
                av_pend = []
                for kt in range(nkt):
                    k0 = kt * TK
                    c0 = max(0, k0 - q0)  # narrowed live query-range start
                    kc, kk = k0 // TQ, k0 % TQ
                    s_ps = ps.tile([128, 2 * TQ], F32, tag="s")
                    _mm(nc, s_ps[:, c0:TQ],
                        qkTs[kc][0:64, 4 + pair, kk : kk + TK], qE[:, c0:TQ])
                    _mm(nc, s_ps[:, TQ + c0 : 2 * TQ],
                        qkTs[kc][64:128, 4 + pair, kk : kk + TK], qO[:, c0:TQ])
                    e_sb = p2.tile([128, 2 * TQ], BF16, tag="e")
                    # e = exp(scores / sqrt(d_k)); no max-subtraction needed:
                    # scores/8 is O(1) for these inputs, exp cannot overflow.
                    s_v = s_ps[:].rearrange("p (h q) -> p h q", h=2)[:, :, c0:TQ]
                    e_v = e_sb[:].rearrange("p (h q) -> p h q", h=2)[:, :, c0:TQ]
                    nc.scalar.activation(
                        out=e_v, in_=s_v,
                        func=mybir.ActivationFunctionType.Exp, scale=0.125)
                    if k0 + TK - 1 > q0:  # diagonal block: zero where k > q
                        rn = min(c0 + TK, TQ)
                        for half in range(2):
                            nc.gpsimd.affine_select(
                                out=e_sb[:, half * TQ + c0 : half * TQ + rn],
                                in_=e_sb[:, half * TQ + c0 : half * TQ + rn],
                                compare_op=mybir.AluOpType.is_ge,
                                fill=0.0, base=q0 + c0 - k0,
                                pattern=[[1, rn - c0]], channel_multiplier=-1)
                    if len(av_pend) >= av_depth:
                        av_mms(*av_pend.pop(0))
                    av_pend.append((e_sb, kt, c0))
                    ktg += 1
                    # one pending PE work item per kt: chunk work first (it
                    # has a hard deadline), then projection work once this
                    # qt's early norms have certainly landed.
                    if q_chunk:
                        q_chunk.pop(0)()
                        if len(q_chunk) > 12:  # backlog: the queue must drain
                            q_chunk.pop(0)()   # before the next qt needs it
                    elif q_proj and ktg >= 8:
                        q_proj.pop(0)()
                for a in av_pend:
                    av_mms(*a)
                make_norm(pair, q0, av_E, av_O,
                          pe_bcast=(qt == NQT - 1 and pair == 0))()
            q_proj.extend(make_proj(
                q0, ko_order=(1, 2, 3, 0) if qt == NQT - 1 else (0, 1, 2, 3)))
        for f in q_chunk:
            f()
        for f in q_proj:
            f()
    nc.finalize()
    return nc


_CACHE = threading.local()


def _get_program():
    nc = getattr(_CACHE, "nc", None)
    if nc is None:
        nc = build_program()
        _CACHE.nc = nc
    return nc


def _make_in_maps(x, W_qkv, b_qkv, W_out, b_out):
    x = np.asarray(x, np.float32)
    W_qkv = np.asarray(W_qkv, np.float32)
    b_qkv = np.asarray(b_qkv, np.float32)
    W_out = np.asarray(W_out, np.float32)
    in_maps = []
    for c in range(NCORES):
        b, g = c // 2, c % 2
        sl = slice(512 * g, 512 * g + 512)  # this head group's q (and k,v) cols
        bf16 = ml_dtypes.bfloat16
        in_maps.append({
            "xT": np.ascontiguousarray(x[b].T.astype(bf16)),
            "wqk": np.ascontiguousarray(
                np.concatenate([W_qkv[:, 0:1024][:, sl], W_qkv[:, 1024:2048][:, sl]],
                               axis=1).astype(bf16)),
            "bqk": np.ascontiguousarray(
                np.concatenate([b_qkv[0:1024][sl], b_qkv[1024:2048][sl]])),
            "wv": np.ascontiguousarray(W_qkv[:, 2048:3072][:, sl].astype(bf16)),
            "bv": np.ascontiguousarray(b_qkv[2048:3072][sl]),
            "wo": np.ascontiguousarray(W_out[sl, :].astype(bf16)),
        })
    return in_maps


def _run(inputs, trace=False):
    nc = _get_program()
    in_maps = _make_in_maps(**inputs)
    res = run_bass_kernel_spmd(nc, in_maps, list(range(NCORES)), trace=trace)
    b_out = np.asarray(inputs["b_out"], np.float32)
    y = np.empty((B, T, C), np.float32)
    for b in range(B):
        yt = (res.results[2 * b]["yT"].astype(np.float32)
              + res.results[2 * b + 1]["yT"].astype(np.float32))
        y[b] = yt.T + b_out
    return y, res


def kernel(x, W_qkv, b_qkv, W_out, b_out):
    y, _ = _run(dict(x=x, W_qkv=W_qkv, b_qkv=b_qkv, W_out=W_out, b_out=b_out))
    return y



# revision 39
# speedup vs baseline: 1.1949x; 1.1949x over previous
"""Causal self-attention (B=4, T=2048, C=1024, H=16) on 8 trn2 NeuronCores.

Sharding: tensor-parallel over heads x data-parallel over batch.
Core c handles batch b=c//2 and head group g=c%2 (8 heads each).
Each core computes qkv projection for its heads, causal attention, and a
partial output projection; the host sums the two partial yT per batch and
adds the output bias.

Device dataflow is feature-major ("transposed") end to end:
  qkT[f, t]   = Wqk.T @ xT          (f = head-pair-blocked q/k features)
  scoresT[k, q] = kT.T @ qT         per head, k-tile=128 x q-tile=512
  e = exp(scoresT/8), causal-masked via affine_select
  avT[d(+1), q] += [v|1].T @ e      ones-column gives softmax denominator
  aoT = avT[0:64] * (1/avT[64]) broadcast (PE outer-product broadcast)
  yT_partial = Wo.T @ aoT
No transposes are needed anywhere; the host transposes x and y (free).
Heads are packed two per 128-partition block (even head at partitions 0-63,
odd at 64-127) so the K=64 score matmuls of a pair run row-tiled
concurrently in the PE array.
"""

import os
import threading
from contextlib import ExitStack

import ml_dtypes
import numpy as np

import concourse.bass as bass
from concourse import bacc
import concourse.mybir as mybir
import concourse.tile as tile
from concourse.bass_utils import run_bass_kernel_spmd

B, T, C = 4, 2048, 1024
H, D = 16, 64
NCORES = 8
HL = 8                 # heads per core
NPAIR = HL // 2        # head pairs per core
CQK = 2 * HL * D       # 1024 local q+k features
CV = HL * D            # 512 local v features
TQ = 512               # query tile (PSUM bank limit for f32)
NQT = T // TQ          # 4
TK = 128               # key tile (PSUM partition limit)
NKT = T // TK          # 16
KO = C // 128          # 8 contraction tiles over C
F32 = mybir.dt.float32
BF16 = mybir.dt.bfloat16

# float32r: full-precision fp32 data, fast PE streaming mode (1 cycle/row at
# N>=256 vs 4 for plain float32).
MM_DT = {
    "f32r": mybir.dt.float32r,
    "f32": mybir.dt.float32,
}[os.environ.get("ATTN_MM_DT", "f32r")]


def r(ap):
    """View an fp32 AP as the matmul input dtype (float32r needs producers to
    write through an fp32r-typed AP so the BIR verifier sees rounded data)."""
    if MM_DT == F32 or ap.dtype != F32:
        return ap
    return ap.bitcast(MM_DT)


def _mm(nc, out, lhsT, rhs, start=True, stop=True):
    nc.tensor.matmul(out, r(lhsT), r(rhs), start=start, stop=stop)


def build_program():
    nc = bacc.Bacc(None)
    xT = nc.declare_dram_parameter("xT", [C, T], BF16, isOutput=False)
    wqk = nc.declare_dram_parameter("wqk", [C, CQK], BF16, isOutput=False)
    bqk = nc.declare_dram_parameter("bqk", [CQK], F32, isOutput=False)
    wv = nc.declare_dram_parameter("wv", [C, CV], BF16, isOutput=False)
    bv = nc.declare_dram_parameter("bv", [CV], F32, isOutput=False)
    wo = nc.declare_dram_parameter("wo", [CV, C], BF16, isOutput=False)
    yT = nc.declare_dram_parameter("yT", [C, T], BF16, isOutput=True)

    with ExitStack() as ctx:
        ctx.enter_context(nc.allow_low_precision(reason="fp32r matmul inputs"))
        tc = ctx.enter_context(tile.TileContext(nc))
        persist = ctx.enter_context(tc.tile_pool(name="persist", bufs=1))
        p2 = ctx.enter_context(tc.tile_pool(name="p2", bufs=3))
        pw = ctx.enter_context(tc.tile_pool(name="pw", bufs=1))
        px = ctx.enter_context(tc.tile_pool(name="px", bufs=2))
        ps = ctx.enter_context(tc.tile_pool(name="ps", bufs=2, space="PSUM"))
        ps_av = ctx.enter_context(tc.tile_pool(name="ps_av", bufs=2, space="PSUM"))
        dram = ctx.enter_context(tc.tile_pool(name="dram", bufs=2, space="DRAM"))

        # q/k features, head-pair blocked: block m<4 = q of pair m
        # (even head partitions 0-63, odd 64-127), block 4+m = k of pair m.
        # One tile per 512-token chunk so chunk writes and attention reads
        # of different chunks never false-serialize (deps are per-tile).
        qkTs = [persist.tile([128, 8, TQ], BF16, name=f"qkT{c}")
                for c in range(NQT)]
        # v with ones column for the softmax denominator: [tok, kt, head, d+1]
        v_augs = [persist.tile([128, TQ // TK, HL, D + 1], BF16,
                               name=f"vaug{c}") for c in range(NQT)]
        bqk_sb = persist.tile([128, 8], F32)
        bv_row = persist.tile([1, CV], F32)
        bvb_sb = persist.tile([128, CV], F32)    # v bias broadcast over tokens
        ones_sb = persist.tile([128, 128], F32)
        wo_sb = persist.tile([128, 4, C], BF16)
        # normalized attention output, one tile per head pair (per-tile deps:
        # the projection's per-ko reads then only wait on that pair's norm)
        aoTs = [persist.tile([128, T], BF16, name=f"aoT{p}")
                for p in range(NPAIR)]

        ones_f32 = persist.tile([128, 128], F32)
        nc.vector.memset(ones_f32, 1.0)
        nc.vector.tensor_copy(out=r(ones_sb[:]), in_=ones_f32)
        for c in range(NQT):
            nc.vector.tensor_copy(
                out=v_augs[c][:, :, :, D : D + 1],
                in_=ones_f32[:, 0 : (TQ // TK) * HL].rearrange(
                    "p (a b c) -> p a b c", a=TQ // TK, b=HL))
        nc.sync.dma_start(out=bqk_sb, in_=bqk[:].rearrange("(m p) -> p m", p=128))
        nc.sync.dma_start(out=r(bv_row[:]), in_=r(bv[:].unsqueeze(0)))

        xT_r = xT[:].rearrange("(ko p) t -> p ko t", p=128)
        wv_r = wv[:].rearrange("(ko p) f -> p ko f", p=128)
        wqk_r = wqk[:].rearrange("(ko p) f -> p ko f", p=128)
        # chunk-0 x and the v weights load first (ko-halves for finer deps)
        # so the first v matmuls start as early as possible.
        KH = KO // 2
        xt0 = [px.tile([128, KH, TQ], BF16, name=f"xt0_{h}", tag=f"xt{h}")
               for h in range(2)]
        wv_sb = [pw.tile([128, KH, CV], BF16, name=f"wv_{h}", tag=f"wv{h}")
                 for h in range(2)]
        # two parallel DMA queues for the startup loads. The attention
        # critical chain needs the first pair's q/k blocks = xt (sync queue)
        # + wqk (gpsimd queue), both in ko-halves, so the first q/k matmuls
        # start at ~1MB queue depth and the full chain (qk -> bias ->
        # scores -> exp -> av) is rolling by ~10us. wv lands last on sync;
        # the v matmuls fill the PE behind the early attention steps.
        wqk_sb = [pw.tile([128, KH, CQK], BF16, name=f"wqk_{h}", tag=f"wqk{h}")
                  for h in range(2)]
        for h in range(2):
            nc.sync.dma_start(out=r(xt0[h][:]),
                              in_=r(xT_r[:, h * KH : (h + 1) * KH, 0:TQ]))
            nc.gpsimd.dma_start(out=r(wqk_sb[h][:]),
                                in_=r(wqk_r[:, h * KH : (h + 1) * KH, :]))
        for h in range(2):
            nc.sync.dma_start(out=r(wv_sb[h][:]),
                              in_=r(wv_r[:, h * KH : (h + 1) * KH, :]))

        # v-bias broadcast over the 128 token partitions via K=1 outer product
        bvb_ps = ps.tile([128, CV], F32, tag="s")
        _mm(nc, bvb_ps, ones_sb[0:1, :], bv_row)
        nc.vector.tensor_copy(out=bvb_sb, in_=bvb_ps)

        def qkv_chunk_items(ch, xt, split_v=False):
            """Per-chunk QKV work, as one closure per matmul group."""
            t0 = ch * TQ

            def v_mt(mt, kos=range(KO), acc_in=None):
                def f():
                    acc = acc_in or ps.tile([128, CV], F32, tag="s")
                    for ko in kos:
                        _mm(nc, acc,
                            xt[ko // KH][:, ko % KH, mt * TK : (mt + 1) * TK],
                            wv_sb[ko // KH][:, ko % KH, :],
                            start=ko == 0, stop=ko == KO - 1)
                    if kos[-1] == KO - 1:
                        nc.vector.tensor_add(
                            out=v_augs[ch][:, mt, :, 0:D],
                            in0=acc.rearrange("p (h d) -> p h d", d=D),
                            in1=bvb_sb.rearrange("p (h d) -> p h d", d=D))
                    return acc
                return f

            def qk_m(m, kos=range(KO), acc_in=None):
                def f():
                    acc = acc_in or ps.tile([128, TQ], F32, tag="s")
                    for ko in kos:
                        _mm(nc, acc,
                            wqk_sb[ko // KH][:, ko % KH, m * 128 : (m + 1) * 128],
                            xt[ko // KH][:, ko % KH, :],
                            start=ko == 0, stop=ko == KO - 1)
                    if kos[-1] == KO - 1:
                        nc.vector.tensor_scalar_add(
                            out=qkTs[ch][:, m, :], in0=acc,
                            scalar1=bqk_sb[:, m : m + 1])
                    return acc
                return f

            if split_v:
                # chunk-0 prologue: half-contraction interleave (A = ko
                # h0 halves, which arrive first). The first pair's q/k
                # blocks (0 and 4) go first: they start the attention
                # critical chain. v fills the PE behind them. At most two
                # accumulation groups are open at any point (the PSUM "s"
                # pool has two slots).
                accs = {}
                A, Bk = list(range(KH)), list(range(KH, KO))
                def gA(mk, i):
                    return lambda: accs.__setitem__((mk, i), (qk_m if mk == "q"
                        else v_mt)(i, kos=A)())
                def gB(mk, i):
                    return lambda: (qk_m if mk == "q" else v_mt)(
                        i, kos=Bk, acc_in=accs[(mk, i)])()
                seq = [("q", 0), ("q", 4), ("q", 0), ("q", 4),
                       ("v", 0), ("v", 0), ("v", 1), ("v", 1),
                       ("v", 2), ("v", 2), ("v", 3), ("v", 3)]
                seen = set()
                items = []
                for mk, i in seq:
                    if (mk, i) in seen:
                        items.append(gB(mk, i))
                    else:
                        seen.add((mk, i))
                        items.append(gA(mk, i))
                return items, [qk_m(m) for m in (1, 5, 2, 6, 3, 7)]
            return [v_mt(mt) for mt in range(TQ // TK)] + \
                   [qk_m(m) for m in range(8)]

        def load_chunk(ch):
            xt = [px.tile([128, KH, TQ], BF16, name=f"xt_{ch}_{h}", tag=f"xt{h}")
                  for h in range(2)]
            t0 = ch * TQ
            for h in range(2):
                nc.sync.dma_start(
                    out=r(xt[h][:]),
                    in_=r(xT_r[:, h * KH : (h + 1) * KH, t0 : t0 + TQ]))
            return xt

        # chunk 0 prologue: the first pair's q/k blocks and v run dense;
        # the other six q/k blocks spread into the attention stream (pair p
        # of qt0 only needs blocks p and 4+p, which land pairs ahead).
        pre0, rest0 = qkv_chunk_items(0, xt0, split_v=True)
        for f in pre0:
            f()
        # out-proj weights are not needed until much later; load them now so
        # the DMA does not compete with the startup x/wv/wqk loads.
        nc.sync.dma_start(out=wo_sb, in_=wo[:].rearrange("(ko p) f -> p ko f", p=128))

        def make_norm(pair, q0, av_E, av_O, pe_bcast=False):
            def norm():
                if pe_bcast:
                    # tail variant: broadcast denominators with a K=1 PE outer
                    # product (PE is idle here) instead of the DRAM bounce,
                    # skipping two DMA-completion latencies.
                    stage = p2.tile([128, 2 * TQ], F32, tag="rec", bufs=2)
                    nc.vector.tensor_copy(out=r(stage[64:65, 0:TQ]),
                                          in_=av_E[D : D + 1, :])
                    nc.vector.tensor_copy(out=r(stage[64:65, TQ : 2 * TQ]),
                                          in_=av_O[D : D + 1, :])
                    bc_ps = ps.tile([64, 2 * TQ], F32, tag="s")
                    _mm(nc, bc_ps[:, 0:TQ], ones_sb[64:65, 0:64],
                        stage[64:65, 0:TQ])
                    _mm(nc, bc_ps[:, TQ : 2 * TQ], ones_sb[64:65, 0:64],
                        stage[64:65, TQ : 2 * TQ])
                    bc_sb = p2.tile([64, 2 * TQ], F32, tag="recbc", bufs=2)
                    nc.vector.reciprocal_approx_fast(out=bc_sb, in_=bc_ps)
                    nc.vector.tensor_mul(
                        out=aoTs[pair][0:64, q0 : q0 + TQ],
                        in0=av_E[0:D, :], in1=bc_sb[:, 0:TQ])
                    ao_tmp = p2.tile([64, TQ], BF16, tag="aotmp")
                    nc.vector.tensor_mul(out=ao_tmp, in0=av_O[0:D, :],
                                         in1=bc_sb[:, TQ : 2 * TQ])
                    nc.sync.dma_start(out=aoTs[pair][64:128, q0 : q0 + TQ],
                                      in_=ao_tmp)
                    return
                # denominators (av row D) -> SBUF -> DRAM -> 0-step-partition
                # DMA fans them over the 64 d-partitions; the reciprocal then
                # runs 64-lane-parallel at partition 0 (reciprocal_approx_fast
                # misbehaves at base partition 64).
                stage = p2.tile([128, 2 * TQ], F32, tag="rec", bufs=2)
                nc.vector.tensor_copy(out=stage[64:65, 0:TQ],
                                      in_=av_E[D : D + 1, :])
                nc.vector.tensor_copy(out=stage[64:65, TQ : 2 * TQ],
                                      in_=av_O[D : D + 1, :])
                dr = dram.tile([1, 2 * TQ], F32, tag="drrec")
                nc.sync.dma_start(out=dr, in_=stage[64:65, :])
                den_bc = p2.tile([64, 2 * TQ], F32, tag="bc", bufs=2)
                nc.sync.dma_start(out=den_bc,
                                  in_=dr[:].to_broadcast([64, 2 * TQ]))
                bc_sb = p2.tile([64, 2 * TQ], F32, tag="recbc", bufs=2)
                nc.vector.reciprocal_approx_fast(out=bc_sb, in_=den_bc)
                nc.vector.tensor_mul(
                    out=aoTs[pair][0:64, q0 : q0 + TQ],
                    in0=av_E[0:D, :], in1=bc_sb[:, 0:TQ])
                ao_tmp = p2.tile([64, TQ], BF16, tag="aotmp")
                nc.vector.tensor_mul(out=ao_tmp, in0=av_O[0:D, :],
                                     in1=bc_sb[:, TQ : 2 * TQ])
                # odd head lives at partitions 64-127: DMA does the hop
                nc.sync.dma_start(out=aoTs[pair][64:128, q0 : q0 + TQ],
                                  in_=ao_tmp)
            return norm

        def make_proj(q0, ko_order=(0, 1, 2, 3)):
            def proj_m(m):
                def f():
                    acc = ps.tile([128, TQ], F32, tag="s")
                    for i, ko in enumerate(ko_order):
                        _mm(nc, acc, wo_sb[:, ko, m * 128 : (m + 1) * 128],
                            aoTs[ko][:, q0 : q0 + TQ], start=i == 0, stop=i == 3)
                    y_sb = p2.tile([128, TQ], BF16, tag="ysb", bufs=2)
                    nc.vector.tensor_copy(out=y_sb, in_=acc)
                    nc.sync.dma_start(
                        out=yT[m * 128 : (m + 1) * 128, q0 : q0 + TQ], in_=y_sb)
                return f
            return [proj_m(m) for m in range(8)]

        # Pending PE work spread one item per kt into the ACT-paced attention
        # stream: next chunk's QKV groups (deadline: before the next q-tile)
        # and the previous q-tile's projection (needs this qt's norms done).
        q_chunk = list(rest0)
        q_proj = []

        for qt in range(NQT):
            q0 = qt * TQ
            nkt = (q0 + TQ) // TK  # causal: only k-tiles with k0 <= q0+TQ-1
            if qt + 1 < NQT:
                q_chunk.extend(qkv_chunk_items(qt + 1, load_chunk(qt + 1)))
            ktg = 0
            pair_order = (1, 2, 3, 0) if qt == NQT - 1 else range(NPAIR)
            for pair in pair_order:
                qE = qkTs[qt][0:64, pair, :]
                qO = qkTs[qt][64:128, pair, :]
                av_E = ps_av.tile([D + 1, TQ], F32, tag="avE")
                av_O = ps_av.tile([D + 1, TQ], F32, tag="avO")

                def av_mms(e_sb, kt, c0):
                    vc, vk = kt // (TQ // TK), kt % (TQ // TK)
                    _mm(nc, av_E[:, c0:TQ],
                        v_augs[vc][:, vk, 2 * pair, :], e_sb[:, c0:TQ],
                        start=kt == 0, stop=kt == nkt - 1)
                    _mm(nc, av_O[:, c0:TQ],
                        v_augs[vc][:, vk, 2 * pair + 1, :],
                        e_sb[:, TQ + c0 : 2 * TQ],
                        start=kt == 0, stop=kt == nkt - 1)

                # av matmuls deferred so the next scores sit ahead of
                # av(kt) in the PE queue: the PE computes scores while ACT
                # exps the previous block. Two kts of slack in the late
                # q-tiles, where there is little other PE work per kt and
                # one kt of slack is shorter than the exp latency.
                av_depth = 2 if qt >= 2 else 1
                av_pend = []
                for kt in range(nkt):
                    k0 = kt * TK
                    c0 = max(0, k0 - q0)  # narrowed live query-range start
                    kc, kk = k0 // TQ, k0 % TQ
                    s_ps = ps.tile([128, 2 * TQ], F32, tag="s")
                    _mm(nc, s_ps[:, c0:TQ],
                        qkTs[kc][0:64, 4 + pair, kk : kk + TK], qE[:, c0:TQ])
                    _mm(nc, s_ps[:, TQ + c0 : 2 * TQ],
                        qkTs[kc][64:128, 4 + pair, kk : kk + TK], qO[:, c0:TQ])
                    e_sb = p2.tile([128, 2 * TQ], BF16, tag="e")
                    # e = exp(scores / sqrt(d_k)); no max-subtraction needed:
                    # scores/8 is O(1) for these inputs, exp cannot overflow.
                    s_v = s_ps[:].rearrange("p (h q) -> p h q", h=2)[:, :, c0:TQ]
                    e_v = e_sb[:].rearrange("p (h q) -> p h q", h=2)[:, :, c0:TQ]
                    nc.scalar.activation(
                        out=e_v, in_=s_v,
                        func=mybir.ActivationFunctionType.Exp, scale=0.125)
                    if k0 + TK - 1 > q0:  # diagonal block: zero where k > q
                        rn = min(c0 + TK, TQ)
                        for half in range(2):
                            nc.gpsimd.affine_select(
                                out=e_sb[:, half * TQ + c0 : half * TQ + rn],
                                in_=e_sb[:, half * TQ + c0 : half * TQ + rn],
                                compare_op=mybir.AluOpType.is_ge,
                                fill=0.0, base=q0 + c0 - k0,
                                pattern=[[1, rn - c0]], channel_multiplier=-1)
                    if len(av_pend) >= av_depth:
                        av_mms(*av_pend.pop(0))
                    av_pend.append((e_sb, kt, c0))
                    ktg += 1
                    # one pending PE work item per kt: chunk work first (it
                    # has a hard deadline), then projection work once this
                    # qt's early norms have certainly landed.
                    if q_chunk:
                        q_chunk.pop(0)()
                        if len(q_chunk) > 12:  # backlog: the queue must drain
                            q_chunk.pop(0)()   # before the next qt needs it
                    elif q_proj and ktg >= 8:
                        q_proj.pop(0)()
                for a in av_pend:
                    av_mms(*a)
                make_norm(pair, q0, av_E, av_O,
                          pe_bcast=(qt == NQT - 1 and pair == 0))()
            q_proj.extend(make_proj(
                q0, ko_order=(1, 2, 3, 0) if qt == NQT - 1 else (0, 1, 2, 3)))
        for f in q_chunk:
            f()
        for f in q_proj:
            f()
    nc.finalize()
    return nc


_CACHE = threading.local()


def _get_program():
    nc = getattr(_CACHE, "nc", None)
    if nc is None:
        nc = build_program()
        _CACHE.nc = nc
    return nc


def _make_in_maps(x, W_qkv, b_qkv, W_out, b_out):
    x = np.asarray(x, np.float32)
    W_qkv = np.asarray(W_qkv, np.float32)
    b_qkv = np.asarray(b_qkv, np.float32)
    W_out = np.asarray(W_out, np.float32)
    in_maps = []
    for c in range(NCORES):
        b, g = c // 2, c % 2
        sl = slice(512 * g, 512 * g + 512)  # this head group's q (and k,v) cols
        bf16 = ml_dtypes.bfloat16
        in_maps.append({
            "xT": np.ascontiguousarray(x[b].T.astype(bf16)),
            "wqk": np.ascontiguousarray(
                np.concatenate([W_qkv[:, 0:1024][:, sl], W_qkv[:, 1024:2048][:, sl]],
                               axis=1).astype(bf16)),
            "bqk": np.ascontiguousarray(
                np.concatenate([b_qkv[0:1024][sl], b_qkv[1024:2048][sl]])),
            "wv": np.ascontiguousarray(W_qkv[:, 2048:3072][:, sl].astype(bf16)),
            "bv": np.ascontiguousarray(b_qkv[2048:3072][sl]),
            "wo": np.ascontiguousarray(W_out[sl, :].astype(bf16)),
        })
    return in_maps


def _run(inputs, trace=False):
    nc = _get_program()
    in_maps = _make_in_maps(**inputs)
    res = run_bass_kernel_spmd(nc, in_maps, list(range(NCORES)), trace=trace)
    b_out = np.asarray(inputs["b_out"], np.float32)
    y = np.empty((B, T, C), np.float32)
    for b in range(B):
        yt = (res.results[2 * b]["yT"].astype(np.float32)
              + res.results[2 * b + 1]["yT"].astype(np.float32))
        y[b] = yt.T + b_out
    return y, res


def kernel(x, W_qkv, b_qkv, W_out, b_out):
    y, _ = _run(dict(x=x, W_qkv=W_qkv, b_qkv=b_qkv, W_out=W_out, b_out=b_out))
    return y

